# revision 25
# baseline (speedup 1.0000x reference)
"""Trainium2 Bass kernel: 4x EmbeddingBag(sum over 32 codes) + 3-layer MLP.

Data-parallel over 8 NeuronCores (batch 16384 -> 8 x 2048).  Embedding tables
are concatenated (proc offset by +100000), cast to bf16 and split into 5
chunks of <=32000 rows (so per-chunk row indices fit int16 for dma_gather),
each chunk followed by one zero row used as gather padding.  Table rows are
assigned to chunks by a quota-balancing greedy (plus a repair pass) so that
every (core, window, bag) cell's per-chunk lookup counts stay under rotated
multiples-of-128 quotas - minimizing the ceil-128 padding below.

Per core the 262144 lookups (4 bags x 2048 examples x 32 codes) are sorted by
(window of 128 examples, chunk, bag).  Each (win,ck,bag) segment is padded to
a multiple of 128 rows ("blocks") with zero-row fetches.  One dma_gather per
(win, ck) pulls all its blocks' rows (bf16, 256B each) into SBUF in
partition-fastest order.  Per block, the selection matrix E [128 rows x 128
examples] is generated ON-CHIP by the vector engine (tensor_scalar is_equal
of a constant iota row-tile against the block's per-row example ids "mids",
a per-window fp32 DMA), then one PE matmul per block accumulates the rows
into a per-(bag, win) PSUM tile [D=128, 128 examples] in fp32 - start=True on
the first block of each (win,bag), stop on the last.  Pad rows fetch the
chunk's zero row and carry a sentinel mid (no E column), so they add zero.

The MLP then runs per window in fp32: bag sums are already feature-major in
PSUM, copy to SBUF (ACT/DVE), layer1+layer2 feature-major (ACT applies
bias/relu on the PSUM->SBUF copy), layer3 uses the activations as lhsT to
emit example-major [128, 1000] directly (bias via a K=1 ones-row matmul PSUM
init), ACT sigmoid, output DMA'd bf16 on the ACT HWDGE queue (host converts
to fp32).  Weight preloads are emitted after window 0's gathers; the last two
windows' gather ops are split so the drain tail overlaps remaining gathers.

The Bass program structure is shared by all 8 cores (SPMD); per-op sizes are
the max over cores, deficit cores pad with zero-row gathers and sentinel mids.
"""

import numpy as np

B, L, D = 16384, 32, 128
DIAG_LEN, PROC_LEN, MED_LEN = 100000, 50000, 1000
N_CORES = 8
P = 128
CS = 32000          # chunk size (int16-addressable)
NCK = 5             # chunks
WIN = 128           # examples per window
SENT = 200.0        # mids sentinel (never equals iota 0..127; exact in bf16)
QUOTA = (7, 7, 7, 6, 6)  # per-(win,bag) chunk quotas in 128-blocks, rotated


def _balance_chunks(rows_all, cell_all, v_cat, nwin):
    """Assign table rows to chunks so that per-(core,win,bag) chunk counts
    stay under rotated 128-multiple quotas (minimizing ceil-128 padding).

    rows_all/cell_all: per-lookup row id and cell id (c*nwin*4 + w*4 + b).
    Returns (asg [v_cat] chunk id, loc [v_cat] position within chunk).
    """
    n_cells = cell_all.max() + 1
    o = np.argsort(rows_all, kind="stable")
    rs, cells_s = rows_all[o], cell_all[o]
    row_start = np.searchsorted(rs, np.arange(v_cat + 1))
    cnts = np.diff(row_start)

    # per-(row, cell) occurrence counts, row-major
    key = rs * n_cells + cells_s
    ukey, uocc = np.unique(key, return_counts=True)
    urow = ukey // n_cells
    ucell = ukey % n_cells
    ustart = np.searchsorted(urow, np.arange(v_cat + 1))

    # quotas per (ck, cell): rotate QUOTA by (w*4+b) % NCK
    j = np.arange(n_cells) % (nwin * 4)
    q = np.array(QUOTA, np.int64) * P
    Q = np.empty((NCK, n_cells), np.int64)
    for ck in range(NCK):
        Q[ck] = q[(ck + j) % NCK]

    L = np.zeros((NCK, n_cells), np.int64)
    cap = np.full(NCK, CS, np.int64)
    asg = np.full(v_cat, -1, np.int64)
    # effective ceiling per (ck, cell): quota, ratcheted up by 128 whenever a
    # cell in the same (w,b,ck) group has already overflowed past it (the
    # extra block is paid once per group; later rows fill it for free).
    wb = j  # cell -> (w*4+b) group id
    n_grp = nwin * 4
    C = Q.copy()

    row_order = np.argsort(-cnts, kind="stable")
    BS = 512
    for i0 in range(0, v_cat, BS):
        br = row_order[i0 : i0 + BS]
        # flatten this batch's (row, cell, occ) entries
        ent_s = ustart[br]
        ent_e = ustart[br + 1]
        ent_n = ent_e - ent_s
        flat = np.concatenate([np.arange(s, e) for s, e in zip(ent_s, ent_e)]) \
            if ent_n.sum() else np.empty(0, np.int64)
        bounds = np.concatenate([[0], np.cumsum(ent_n)])
        bcell = ucell[flat]
        bocc = uocc[flat]
        # slack per (row-in-batch, ck) = min over row's cells of C-L-occ
        nb = br.size
        slack = np.full((NCK, nb), 1 << 30, np.int64)
        has = ent_n > 0
        red_idx = bounds[:-1][has]
        for ck in range(NCK):
            cs_ = C[ck, bcell] - L[ck, bcell] - bocc
            if red_idx.size:
                slack[ck, has] = np.minimum.reduceat(cs_, red_idx)
            slack[ck, ~has] = 1 << 30
            if cap[ck] <= 0:
                slack[ck, :] = -(1 << 30)
        choice = np.argmax(slack, axis=0)
        asg[br] = choice
        for ck in range(NCK):
            sel = choice == ck
            cap[ck] -= int(sel.sum())
            csel = np.repeat(sel, ent_n)
            np.add.at(L[ck], bcell[csel], bocc[csel])
            # ratchet ceilings: group ceiling = max over its cells of
            # ceil128(load), at least the quota
            gmax = np.zeros(n_grp, np.int64)
            np.maximum.at(gmax, wb, L[ck])
            gceil = -(-gmax // P) * P
            C[ck] = np.maximum(Q[ck], gceil[wb])

    # repair pass: groups (w,b,ck) barely over a 128 boundary -> move rows
    # contributing to the over-boundary cores into chunks with slack
    cell_rows_order = np.argsort(ucell, kind="stable")
    cell_start = np.searchsorted(ucell[cell_rows_order], np.arange(n_cells + 1))
    for _ in range(2):
        gmaxs = np.zeros((NCK, n_grp), np.int64)
        for ck in range(NCK):
            np.maximum.at(gmaxs[ck], wb, L[ck])
        over = gmaxs % P
        order = np.argsort(np.where(over > 0, over, 1 << 30).reshape(-1))
        moved = 0
        for flatg in order:
            ck, g = divmod(int(flatg), n_grp)
            exc = int(over[ck, g])
            if exc == 0 or exc > 48:
                break
            floor_l = gmaxs[ck, g] - exc
            # offending cells of this group
            gcells = np.nonzero(wb == g)[0]
            bad = gcells[L[ck, gcells] > floor_l]
            for cell in bad:
                need = int(L[ck, cell] - floor_l)
                ent = cell_rows_order[cell_start[cell] : cell_start[cell + 1]]
                cand = ent[asg[urow[ent]] == ck]
                # smallest contributors first
                cand = cand[np.argsort(uocc[cand], kind="stable")]
                for e in cand:
                    if need <= 0:
                        break
                    r = int(urow[e])
                    es, ee = int(ustart[r]), int(ustart[r + 1])
                    rc, ro = ucell[es:ee], uocc[es:ee]
                    for ck2 in range(NCK):
                        if ck2 == ck or cap[ck2] <= 0:
                            continue
                        if np.all(C[ck2, rc] - L[ck2, rc] >= ro):
                            asg[r] = ck2
                            L[ck, rc] -= ro
                            L[ck2, rc] += ro
                            cap[ck] += 1
                            cap[ck2] -= 1
                            need -= int(ro[np.nonzero(rc == cell)[0][0]])
                            moved += 1
                            break
        if moved == 0:
            break

    # positions within chunks (original row order)
    loc = np.zeros(v_cat, np.int64)
    for ck in range(NCK):
        sel = np.nonzero(asg == ck)[0]
        loc[sel] = np.arange(sel.size)
    return asg, loc


def _structure(counts):
    """Static program structure from per-core segment counts.

    counts: [n_cores, NWIN, NCK, 4] lookup counts per (win, ck, bag) segment.
    """
    n_cores, NWIN, NCK, NB_ = counts.shape
    cmax = counts.max(axis=0)  # [NWIN, NCK, 4]
    nb = -(-cmax // P)  # ceil -> blocks per segment
    nb[:, 0, :][nb[:, 0, :] == 0] = 1  # ck0 segments host the start=True matmul
    ops = []
    idx_off = 0
    blk_off = 0
    for w in range(NWIN):
        win_blocks = {bg: [] for bg in range(4)}
        win_ops = []
        for ck in range(NCK):
            op_blocks = []
            for bg in range(4):
                for b in range(int(nb[w, ck, bg])):
                    blk = [bg, False, False]
                    op_blocks.append(blk)
                    win_blocks[bg].append(blk)
            # split the last window's ops so its tail compute overlaps the
            # remaining gather halves (shrinks the end-of-kernel drain)
            parts = 3 if w == NWIN - 1 else (2 if w == NWIN - 2 else 1)
            per = -(-len(op_blocks) // parts)
            for p0 in range(0, len(op_blocks), per):
                pb = op_blocks[p0 : p0 + per]
                n_op = len(pb) * P
                win_ops.append(
                    dict(win=w, ck=ck, idx_off=idx_off, blk_off=blk_off,
                         nb=len(pb), n=n_op, blocks=pb)
                )
                idx_off += n_op
                blk_off += len(pb)
        for bg in range(4):
            assert win_blocks[bg], "every bag needs blocks in every window"
            win_blocks[bg][0][1] = True   # start
            win_blocks[bg][-1][2] = True  # stop
        ops.extend(win_ops)
    return dict(ops=ops, tot_idx=idx_off, tot_blk=blk_off, nb_arr=nb,
                NWIN=NWIN, NCK=NCK)


def host_prep(inputs, n_cores=N_CORES):
    import ml_dtypes

    bf16 = ml_dtypes.bfloat16

    diag = np.asarray(inputs["diag_emb"], np.float32)
    proc = np.asarray(inputs["proc_emb"], np.float32)
    v_diag, d = diag.shape
    tcat = np.concatenate([diag, proc], axis=0)
    v_cat = tcat.shape[0]
    assert NCK * CS >= v_cat

    gl = {
        "cd": np.asarray(inputs["diag_codes"], np.int64),
        "cp": np.asarray(inputs["proc_codes"], np.int64) + v_diag,
        "pd": np.asarray(inputs["prev_diag_codes"], np.int64),
        "pp": np.asarray(inputs["prev_proc_codes"], np.int64) + v_diag,
    }
    b_total, l_codes = gl["cd"].shape
    assert b_total % n_cores == 0
    bc = b_total // n_cores
    assert bc % WIN == 0
    NWIN = bc // WIN

    # per-core flat (row, example, bag) streams
    core_g, core_e, core_bag = [], [], []
    for c in range(n_cores):
        gs, bags = [], []
        for bi, name in enumerate(("cd", "cp", "pd", "pp")):
            g = gl[name][c * bc : (c + 1) * bc].reshape(-1)
            gs.append(g)
            bags.append(np.full(g.size, bi, np.int64))
        core_g.append(np.concatenate(gs))
        core_bag.append(np.concatenate(bags))
        core_e.append(np.tile(np.repeat(np.arange(bc, dtype=np.int64), l_codes), 4))

    # balance rows across chunks to minimize ceil-128 padding
    rows_all = np.concatenate(core_g)
    cell_all = np.concatenate(
        [
            c * (NWIN * 4) + (core_e[c] // WIN) * 4 + core_bag[c]
            for c in range(n_cores)
        ]
    )
    asg, lmap = _balance_chunks(rows_all, cell_all, v_cat, NWIN)

    tbl_dev = np.zeros(((CS + 1) * NCK, d), bf16)
    tbl_dev[asg * (CS + 1) + lmap] = tcat.astype(bf16)

    # flat per-core lookup streams, sorted by (win, ck, bag)
    per_core = []
    counts = np.zeros((n_cores, NWIN, NCK, 4), np.int64)
    for c in range(n_cores):
        g, bag, e = core_g[c], core_bag[c], core_e[c]
        ck = asg[g]
        loc = lmap[g]
        win = e // WIN
        m = e % WIN
        seg = (win * NCK + ck) * 4 + bag
        order = np.argsort(seg, kind="stable")
        per_core.append((seg[order], loc[order], m[order]))
        np.add.at(counts[c].reshape(-1), seg, 1)

    st = _structure(counts)
    TOT_IDX, TOT_B = st["tot_idx"], st["tot_blk"]

    # static per-segment offsets
    seg_sizes = st["nb_arr"].reshape(-1) * P
    seg_off = np.concatenate([[0], np.cumsum(seg_sizes)])[:-1]

    in_maps = []
    iota_np = np.broadcast_to(
        np.arange(P, dtype=np.float32), (P, P)
    ).astype(bf16).copy()
    for c in range(n_cores):
        seg_s, loc_s, m_s = per_core[c]
        pos_in_seg = np.arange(seg_s.size) - np.concatenate(
            [[0], np.cumsum(np.bincount(seg_s, minlength=seg_sizes.size))]
        )[:-1][seg_s]
        pos = seg_off[seg_s] + pos_in_seg
        idx_flat = np.full(TOT_IDX, CS, np.int16)  # pad -> zero row
        idx_flat[pos] = loc_s.astype(np.int16)
        m_flat = np.full(TOT_IDX, SENT, np.float32)
        m_flat[pos] = m_s
        # pack gidx: position i -> [16k + i%16, i//16]
        blk = idx_flat.reshape(TOT_IDX // 16, 16).T
        gidx = np.tile(blk, (8, 1)).copy()
        # mids: position i -> [i%128, i//128], bf16
        mids = np.ascontiguousarray(m_flat.reshape(TOT_B, P).T)
        in_maps.append(dict(tbl=tbl_dev, gidx=gidx, mids=mids, iota=iota_np))

    w1t = np.ascontiguousarray(np.asarray(inputs["W1"], np.float32).T)
    w2t = np.ascontiguousarray(np.asarray(inputs["W2"], np.float32).T)
    w3t = np.ascontiguousarray(np.asarray(inputs["W3"], np.float32).T)
    b1 = np.ascontiguousarray(np.asarray(inputs["b1"], np.float32).reshape(-1, 1))
    b2 = np.ascontiguousarray(np.asarray(inputs["b2"], np.float32).reshape(-1, 1))
    b3 = np.ascontiguousarray(np.asarray(inputs["b3"], np.float32).reshape(1, -1))
    for im in in_maps:
        im.update(w1t=w1t, w2t=w2t, w3t=w3t, b1=b1, b2=b2, b3=b3)

    med = w3t.shape[1]
    cfg = dict(b_core=bc, med=med, v_dev=tbl_dev.shape[0], st=st)
    return in_maps, cfg


def build_nc(cfg):
    import concourse.bass as bass
    import concourse.mybir as mybir
    import concourse.tile as tile
    from concourse import bacc

    f32 = mybir.dt.float32
    bf = mybir.dt.bfloat16
    i16 = mybir.dt.int16
    AF = mybir.ActivationFunctionType
    EQ = mybir.AluOpType.is_equal

    bc, med, v_dev = cfg["b_core"], cfg["med"], cfg["v_dev"]
    st = cfg["st"]
    NWIN, NCK = st["NWIN"], st["NCK"]
    TOT_IDX, TOT_B = st["tot_idx"], st["tot_blk"]
    n_half = med // 2
    assert n_half <= 512

    nc = bacc.Bacc("TRN2", target_bir_lowering=False, debug=False,
                   enable_asserts=False, num_devices=N_CORES)

    tbl = nc.dram_tensor("tbl", [v_dev, D], bf, kind="ExternalInput").ap()
    gidx = nc.dram_tensor("gidx", [P, TOT_IDX // 16], i16, kind="ExternalInput").ap()
    mids = nc.dram_tensor("mids", [P, TOT_B], f32, kind="ExternalInput").ap()
    iota = nc.dram_tensor("iota", [P, P], bf, kind="ExternalInput").ap()
    w1t = nc.dram_tensor("w1t", [2 * D, D], f32, kind="ExternalInput").ap()
    w2t = nc.dram_tensor("w2t", [2 * D, 2 * D], f32, kind="ExternalInput").ap()
    w3t = nc.dram_tensor("w3t", [2 * D, med], f32, kind="ExternalInput").ap()
    b1 = nc.dram_tensor("b1", [D, 1], f32, kind="ExternalInput").ap()
    b2 = nc.dram_tensor("b2", [2 * D, 1], f32, kind="ExternalInput").ap()
    b3 = nc.dram_tensor("b3", [1, med], f32, kind="ExternalInput").ap()
    out = nc.dram_tensor("out", [bc, med], bf, kind="ExternalOutput").ap()

    ops_by_win = {}
    for op in st["ops"]:
        ops_by_win.setdefault(op["win"], []).append(op)

    with tile.TileContext(nc) as tc:
        with (
            tc.tile_pool(name="const", bufs=1) as cpool,
            tc.tile_pool(name="gi", bufs=3) as gi_pool,
            tc.tile_pool(name="mi", bufs=3) as mi_pool,
            tc.tile_pool(name="em", bufs=8) as em_pool,
            tc.tile_pool(name="gath", bufs=8) as gath_pool,
            tc.tile_pool(name="sT", bufs=8) as sT_pool,
            tc.tile_pool(name="acts", bufs=8) as act_pool,
            tc.tile_pool(name="osb", bufs=2) as out_pool,
            tc.tile_pool(name="spsum", bufs=4, space="PSUM") as s_psum,
            tc.tile_pool(name="mpsum", bufs=2, space="PSUM") as m_psum,
            tc.tile_pool(name="opsum", bufs=2, space="PSUM") as o_psum,
        ):
            iota_t = cpool.tile([P, P], bf, tag="iota")
            nc.sync.dma_start(iota_t[:], iota[:, :])

            consts = {}

            def load_consts():
                # Emitted after window 0's gather ops so the first gathers
                # aren't queued behind ~1.3MB of weight preloads.
                ones = cpool.tile([1, P], f32, tag="ones")
                nc.gpsimd.memset(ones[:], 1.0)
                w1t_k = []
                for k in range(2):
                    t = cpool.tile([D, D], f32, tag=f"w1t{k}")
                    nc.sync.dma_start(t[:], w1t[k * D : (k + 1) * D, :])
                    w1t_k.append(t)
                w2t_km = {}
                for k in range(2):
                    for mm in range(2):
                        t = cpool.tile([D, D], f32, tag=f"w2t{k}{mm}")
                        nc.sync.dma_start(
                            t[:], w2t[k * D : (k + 1) * D, mm * D : (mm + 1) * D]
                        )
                        w2t_km[(k, mm)] = t
                w3t_k = []
                for k in range(2):
                    t = cpool.tile([D, med], f32, tag=f"w3t{k}")
                    nc.sync.dma_start(t[:], w3t[k * D : (k + 1) * D, :])
                    w3t_k.append(t)
                b1_t = cpool.tile([D, 1], f32, tag="b1")
                nc.sync.dma_start(b1_t[:], b1[:, :])
                b2_t = []
                for mm in range(2):
                    t = cpool.tile([D, 1], f32, tag=f"b2{mm}")
                    nc.sync.dma_start(t[:], b2[mm * D : (mm + 1) * D, :])
                    b2_t.append(t)
                b3_t = cpool.tile([1, med], f32, tag="b3")
                nc.sync.dma_start(b3_t[:], b3[:, :])
                consts.update(ones=ones, w1t_k=w1t_k, w2t_km=w2t_km,
                              w3t_k=w3t_k, b1_t=b1_t, b2_t=b2_t, b3_t=b3_t)

            for rep in range(cfg.get("reps", 1)):
              for w in range(NWIN):
                s_ps = [s_psum.tile([D, WIN], f32, tag="s", name=f"s{rep}_{w}_{i}") for i in range(4)]
                wops = ops_by_win[w]
                w_idx_off = wops[0]["idx_off"]
                w_blk_off = wops[0]["blk_off"]
                w_n = sum(op["n"] for op in wops)
                w_nb = sum(op["nb"] for op in wops)
                gi = gi_pool.tile([P, w_n // 16], i16, tag="gi")
                if not cfg.get("skip_gi"):
                    nc.sync.dma_start(
                        gi[:],
                        gidx[:, w_idx_off // 16 : (w_idx_off + w_n) // 16],
                    )
                mi = mi_pool.tile([P, w_nb], f32, tag="mi")
                nc.sync.dma_start(
                    mi[:], mids[:, w_blk_off : w_blk_off + w_nb]
                )
                for op in wops:
                    n, nb = op["n"], op["nb"]
                    o16 = (op["idx_off"] - w_idx_off) // 16
                    ob0 = op["blk_off"] - w_blk_off
                    gt = gath_pool.tile([P, nb * D], bf, tag="gath")
                    if not cfg.get("skip_gather"):
                        nc.gpsimd.dma_gather(
                            out_ap=gt[:].rearrange("p (c d) -> p c d", d=D),
                            in_ap=tbl[
                                op["ck"] * (CS + 1) : (op["ck"] + 1) * (CS + 1), :
                            ],
                            idxs_ap=gi[:, o16 : o16 + n // 16],
                            num_idxs=n,
                            num_idxs_reg=n,
                            elem_size=D,
                            single_packet=False,
                        )
                    gt3 = gt[:].rearrange("p (c d) -> p c d", d=D)
                    em = em_pool.tile([P, P * nb], bf, tag="em")
                    for b in range(nb):
                        nc.vector.tensor_scalar(
                            em[:, b * P : (b + 1) * P],
                            iota_t[:],
                            mi[:, ob0 + b : ob0 + b + 1],
                            None,
                            EQ,
                        )
                    if cfg.get("skip_smm"):
                        continue
                    for b, (bg, start, stop) in enumerate(op["blocks"]):
                        nc.tensor.matmul(
                            s_ps[bg][:],
                            lhsT=gt3[:, b, :],
                            rhs=em[:, b * P : (b + 1) * P],
                            start=start,
                            stop=stop,
                            skip_group_check=True,
                        )
                if not consts:
                    load_consts()
                ones = consts["ones"]
                w1t_k, w2t_km = consts["w1t_k"], consts["w2t_km"]
                w3t_k = consts["w3t_k"]
                b1_t, b2_t, b3_t = consts["b1_t"], consts["b2_t"], consts["b3_t"]
                if cfg.get("skip_mlp"):
                    continue
                # bag sums (feature-major) PSUM -> SBUF on ACT
                sT = []
                for bg in range(4):
                    t = sT_pool.tile([D, P], f32, tag="sT", name=f"sT{w}_{bg}")
                    if bg % 2 == 0:
                        nc.scalar.activation(t[:], s_ps[bg][:], AF.Copy)
                    else:
                        nc.vector.tensor_copy(t[:], s_ps[bg][:])
                    sT.append(t)

                l1 = []
                for ka, kb in ((0, 1), (2, 3)):
                    pc = m_psum.tile([P, P], f32, tag="mp")
                    nc.tensor.matmul(
                        pc[:], lhsT=w1t_k[0][:], rhs=sT[ka][:], start=True, stop=False
                    )
                    nc.tensor.matmul(
                        pc[:], lhsT=w1t_k[1][:], rhs=sT[kb][:], start=False, stop=True
                    )
                    xt = act_pool.tile([D, P], f32, tag="l1")
                    nc.scalar.activation(xt[:], pc[:], AF.Identity, bias=b1_t[:])
                    l1.append(xt)

                hT = []
                for mm in range(2):
                    ph = m_psum.tile([P, P], f32, tag="mp")
                    nc.tensor.matmul(
                        ph[:], lhsT=w2t_km[(0, mm)][:], rhs=l1[0][:],
                        start=True, stop=False,
                    )
                    nc.tensor.matmul(
                        ph[:], lhsT=w2t_km[(1, mm)][:], rhs=l1[1][:],
                        start=False, stop=True,
                    )
                    ht = act_pool.tile([D, P], f32, tag="l2")
                    nc.scalar.activation(ht[:], ph[:], AF.Relu, bias=b2_t[mm][:])
                    hT.append(ht)

                ob = out_pool.tile([P, med], bf, tag="osb")
                for h_i in range(2):
                    n0, n1 = h_i * n_half, (h_i + 1) * n_half
                    po = o_psum.tile([P, n_half], f32, tag="op")
                    nc.tensor.matmul(
                        po[:], lhsT=ones[:1, :], rhs=b3_t[:1, n0:n1],
                        start=True, stop=False,
                    )
                    nc.tensor.matmul(
                        po[:], lhsT=hT[0][:], rhs=w3t_k[0][:, n0:n1],
                        start=False, stop=False,
                    )
                    nc.tensor.matmul(
                        po[:], lhsT=hT[1][:], rhs=w3t_k[1][:, n0:n1],
                        start=False, stop=True,
                    )
                    nc.scalar.activation(ob[:, n0:n1], po[:], AF.Sigmoid)
                nc.scalar.dma_start(out[w * P : (w + 1) * P, :], ob[:])

    nc.compile()
    return nc


def kernel(**inputs) -> np.ndarray:
    from concourse.bass_utils import run_bass_kernel_spmd

    in_maps, cfg = host_prep(inputs)
    nc = build_nc(cfg)
    res = run_bass_kernel_spmd(nc, in_maps, core_ids=list(range(N_CORES)))
    return np.concatenate(
        [r["out"].astype(np.float32) for r in res.results], axis=0
    )
